# revision 10
# baseline (speedup 1.0000x reference)
"""Trainium2 Bass kernel: batched self-attention layer (B=4, N=2048, D=512).

Reference (per batch b):
    xt = x[b].T                # x[b] is [D_IN, N] -> xt [N, D_IN]
    Q = xt @ Wq + bq;  K = xt @ Wk + bk;  V = xt @ Wv + bv
    S = Q @ K.T / sqrt(D)
    out[b] = softmax(S, -1) @ V

Sharding: 8 cores, core c -> batch b=c//2, query rows [qh*1024,(qh+1)*1024),
qh=c%2. Each core computes K/V for the full 2048 keys of its batch
(duplicated across the pair; no collectives).

Layout strategy (zero transposes anywhere):
  - x[b] arrives [D_IN, N]: natural lhsT/rhs layouts for everything.
  - Q^T [d, n]  = Wq.T @ x       (lhsT=Wq chunk, rhs=x chunk)
  - K^T [d, n]  = Wk.T @ x
  - V   [m, d]  = x.T @ Wv       (lhsT=x chunk,  rhs=Wv chunk)
  - S^T [m, q]  = K @ Q^T        (lhsT=K^T chunk, rhs=Q^T)
  - E = exp(S^T * scale) on ACT (scores are O(1); no max-subtraction needed)
  - out [q, d]  = E^T.T @ V      (lhsT=E chunk, rhs=V chunk), PSUM-accumulated
  - denom [q,1] = E^T.T @ ones   (same stationary operand, free-dim-1 matmul)
  - out = out * (1/denom) + bv   (bv commutes past normalization since the
    softmax weights sum to 1; bv broadcast tile built once via a K=1 matmul)
All matmul inputs are bitcast to float32r (full-rate PE at free dim >= 256).
"""

import math

import numpy as np

import concourse.bass as bass
import concourse.tile as tile
from concourse import bacc, mybir
from concourse.bass_utils import run_bass_kernel_spmd

P = 128
B, D_IN, N, D_OUT = 4, 512, 2048, 512
NQ = N // 2          # query rows per core
QS = 512             # query rows per attention slice
KC = D_IN // P       # contraction chunks for projections
DC = D_OUT // P      # output-feature chunks
MT = N // P          # key tiles
N_CORES = 8
F32 = mybir.dt.float32
FR = mybir.dt.float32r
BF = mybir.dt.bfloat16
SCALE = 1.0 / math.sqrt(float(D_OUT))

_CACHE = {}


def _emit(tc, nc, d):
    with (
        tc.tile_pool(name="pers", bufs=1) as pers,
        tc.tile_pool(name="work", bufs=3) as work,
        tc.tile_pool(name="ps_proj", bufs=2, space="PSUM") as ps_proj,
        tc.tile_pool(name="ps_sc", bufs=2, space="PSUM") as ps_sc,
        tc.tile_pool(name="ps_out", bufs=4, space="PSUM") as ps_out,
        tc.tile_pool(name="dram", bufs=1, space="DRAM") as dram,
    ):
        # ---- persistent loads ----
        # x arrives host-permuted: this core's query half occupies columns
        # [0, NQ) (so Q-projection slices x directly); key order is permuted,
        # which softmax/out sums are invariant to.
        # DMA order: wq -> x first halves -> wk -> x second halves -> wv, so
        # Q-projection matmuls start ~3us in and overlap the rest of the load.
        w_sb = {}

        def load_w(wn):
            for c in range(KC):
                t = pers.tile([P, D_OUT], BF, tag=f"{wn}{c}", name=f"{wn}{c}")
                nc.sync.dma_start(t, d[wn][:, c, :])
                w_sb[wn, c] = t

        load_w("wk")
        x_sb = []
        for c in range(KC):
            t = pers.tile([P, NQ], BF, tag=f"x{c}", name=f"x{c}")
            nc.sync.dma_start(t, d["x"][:, c, :])
            x_sb.append(t)
        bk_sb = pers.tile([P, KC], F32, tag="bk", name="bk_sb")
        nc.sync.dma_start(bk_sb, d["bk"][:, :])
        load_w("wv")
        load_w("wq")
        bq_sb = pers.tile([P, KC], F32, tag="bq", name="bq_sb")
        nc.sync.dma_start(bq_sb, d["bq"][:, :])
        xq_sb = x_sb
        ones_m = pers.tile([P, 2], BF, tag="ones_m", name="ones_m")
        nc.vector.memset(ones_m, 1.0)
        zeros_8 = pers.tile([P, 8], BF, tag="zeros_8", name="zeros_8")
        nc.vector.memset(zeros_8, 0.0)
        bv_bc = pers.tile([P, D_OUT], F32, tag="bv_bc", name="bv_bc")
        nc.sync.dma_start(bv_bc, d["bvb"][:, :])

        # ---- projections ----
        # K/V are computed for the LOCAL half of the keys only (the same
        # columns as our queries), then pairwise all-gathered with the
        # partner core holding the same batch. Gather output is in
        # canonical [rank0-half, rank1-half] key order on both cores.
        kin = dram.tile([D_IN, NQ], BF, tag="kin", name="kin")
        kout = dram.tile([2 * D_IN, NQ], BF, tag="kout", name="kout")
        vin = dram.tile([NQ, D_OUT], BF, tag="vin", name="vin")
        vout = dram.tile([N, D_OUT], BF, tag="vout", name="vout")
        RG = [[2 * i, 2 * i + 1] for i in range(N_CORES // 2)]

        for i in range(DC):
            for s in range(NQ // 512):
                ps = ps_proj.tile([P, 512], F32, tag="pp", name="ps_k")
                for c in range(KC):
                    nc.tensor.matmul(
                        ps,
                        lhsT=(w_sb["wk", c][:, i * P:(i + 1) * P]),
                        rhs=(x_sb[c][:, s * 512:(s + 1) * 512]),
                        start=(c == 0), stop=(c == KC - 1),
                    )
                kl = work.tile([P, 512], BF, tag="kl", name="kl")
                nc.vector.tensor_scalar_add(kl, ps, bk_sb[:, i:i + 1])
                nc.sync.dma_start(kin[i * P:(i + 1) * P, s * 512:(s + 1) * 512], kl)
        nc.gpsimd.collective_compute(
            "AllGather", mybir.AluOpType.bypass, replica_groups=RG,
            ins=[kin.opt()], outs=[kout.opt()],
        )
        kt_sb = [pers.tile([P, N], BF, tag=f"kt{i}", name=f"kt{i}") for i in range(DC)]
        for i in range(DC):
            for r in range(2):
                nc.sync.dma_start(
                    kt_sb[i][:, r * NQ:(r + 1) * NQ],
                    kout[r * D_IN + i * P:r * D_IN + (i + 1) * P, :],
                )

        for m in range(MT // 2):
            ps = ps_proj.tile([P, D_OUT], F32, tag="pp", name="ps_v")
            for c in range(KC):
                nc.tensor.matmul(
                    ps,
                    lhsT=(x_sb[c][:, m * P:(m + 1) * P]),
                    rhs=(w_sb["wv", c]),
                    start=(c == 0), stop=(c == KC - 1),
                )
            vl = work.tile([P, D_OUT], BF, tag="vl", name="vl")
            nc.vector.tensor_copy(vl, ps)
            nc.sync.dma_start(vin[m * P:(m + 1) * P, :], vl)
        nc.gpsimd.collective_compute(
            "AllGather", mybir.AluOpType.bypass, replica_groups=RG,
            ins=[vin.opt()], outs=[vout.opt()],
        )
        v_sb = [pers.tile([P, D_OUT], BF, tag=f"v{m}", name=f"v{m}") for m in range(MT)]
        for m in range(MT):
            nc.sync.dma_start(v_sb[m], vout[m * P:(m + 1) * P, :])

        qt_sb = [pers.tile([P, NQ], BF, tag=f"qt{i}", name=f"qt{i}") for i in range(DC)]
        for i in range(DC):
            for s in range(NQ // 512):
                ps = ps_proj.tile([P, 512], F32, tag="pp", name="ps_q")
                for c in range(KC):
                    nc.tensor.matmul(
                        ps,
                        lhsT=(w_sb["wq", c][:, i * P:(i + 1) * P]),
                        rhs=(xq_sb[c][:, s * 512:(s + 1) * 512]),
                        start=(c == 0), stop=(c == KC - 1),
                    )
                nc.vector.tensor_scalar_add(
                    qt_sb[i][:, s * 512:(s + 1) * 512], ps, bq_sb[:, i:i + 1]
                )

        # ---- attention ----
        for s in range(NQ // QS):
            outp = [ps_out.tile([P, D_OUT], F32, tag="po", name=f"ps_o{q}") for q in range(QS // P)]
            dnp = ps_proj.tile([P, 2 * (QS // P)], F32, tag="pp", name="ps_dn")
            # one start=True matmul zeroes the whole denominator bank; the
            # per-q groups below accumulate with start=False (a start=True per
            # group would mark the full 2KB zero-region pending-zero and wipe
            # its bank-mates)
            nc.tensor.matmul(
                dnp, lhsT=(kt_sb[0][:, 0:P]), rhs=(zeros_8),
                start=True, stop=False, skip_group_check=True,
            )
            for m in range(MT):
                sps = ps_sc.tile([P, QS], F32, tag="ps", name="ps_s")
                for i in range(DC):
                    nc.tensor.matmul(
                        sps,
                        lhsT=(kt_sb[i][:, m * P:(m + 1) * P]),
                        rhs=(qt_sb[i][:, s * QS:(s + 1) * QS]),
                        start=(i == 0), stop=(i == DC - 1),
                    )
                e_sb = work.tile([P, QS], BF, tag="e", name="e_sb")
                nc.scalar.activation(
                    e_sb, sps, mybir.ActivationFunctionType.Exp, scale=SCALE
                )
                for q in range(QS // P):
                    nc.tensor.matmul(
                        outp[q],
                        lhsT=(e_sb[:, q * P:(q + 1) * P]),
                        rhs=(v_sb[m]),
                        start=(m == 0), stop=(m == MT - 1),
                    )
                    nc.tensor.matmul(
                        dnp[:, 2 * q:2 * q + 2],
                        lhsT=(e_sb[:, q * P:(q + 1) * P]),
                        rhs=(ones_m),
                        start=False, stop=(m == MT - 1),
                        skip_group_check=True,
                    )
            for q in range(QS // P):
                rc = work.tile([P, 1], F32, tag="rc", name="rc")
                nc.vector.reciprocal(rc, dnp[:, 2 * q:2 * q + 1])
                o_sb = work.tile([P, D_OUT], F32, tag="o", name="o_sb")
                nc.vector.scalar_tensor_tensor(
                    o_sb, outp[q], rc, bv_bc,
                    op0=mybir.AluOpType.mult, op1=mybir.AluOpType.add,
                )
                q0 = s * QS + q * P
                nc.sync.dma_start(d["out"][q0:q0 + P, :], o_sb)


def build():
    if "nc" in _CACHE:
        return _CACHE["nc"]
    nc = bacc.Bacc("TRN2", target_bir_lowering=False, debug=False,
                   num_devices=N_CORES)
    d = {
        "x": nc.declare_dram_parameter("x", [P, KC, NQ], BF, isOutput=False),
        "wq": nc.declare_dram_parameter("wq", [P, KC, D_OUT], BF, isOutput=False),
        "wk": nc.declare_dram_parameter("wk", [P, KC, D_OUT], BF, isOutput=False),
        "wv": nc.declare_dram_parameter("wv", [P, KC, D_OUT], BF, isOutput=False),
        "bq": nc.declare_dram_parameter("bq", [P, KC], F32, isOutput=False),
        "bk": nc.declare_dram_parameter("bk", [P, KC], F32, isOutput=False),
        "bvb": nc.declare_dram_parameter("bvb", [P, D_OUT], F32, isOutput=False),
        "out": nc.declare_dram_parameter("out", [NQ, D_OUT], F32, isOutput=True),
    }
    with tile.TileContext(nc) as tc:
        _emit(tc, nc, d)
    nc.compile()
    _CACHE["nc"] = nc
    return nc


def _f32(a):
    return np.ascontiguousarray(np.asarray(a), dtype=np.float32)


def _bf16(a):
    import ml_dtypes
    return np.ascontiguousarray(np.asarray(a, dtype=np.float32).astype(ml_dtypes.bfloat16))


def make_in_maps(x, Wq, bq, Wk, bk, Wv, bv):
    x = _f32(x)
    # [in, out] weights -> [128, 4, 512]: w[p, c, o] = W[c*128+p, o]
    wq = _bf16(_f32(Wq).reshape(KC, P, D_OUT).transpose(1, 0, 2))
    wk = _bf16(_f32(Wk).reshape(KC, P, D_OUT).transpose(1, 0, 2))
    wv = _bf16(_f32(Wv).reshape(KC, P, D_OUT).transpose(1, 0, 2))
    bqh = _f32(bq).reshape(KC, P).T.copy()   # [128, 4]: b[c*128+p] at [p, c]
    bkh = _f32(bk).reshape(KC, P).T.copy()
    bvb = np.tile(_f32(bv).reshape(1, D_OUT), (P, 1)).copy()
    in_maps = []
    for c in range(N_CORES):
        b, qh = divmod(c, 2)
        xb = x[b][:, qh * NQ:(qh + 1) * NQ]               # local half [512, 1024]
        xh = _bf16(xb.reshape(KC, P, NQ).transpose(1, 0, 2))
        in_maps.append({
            "x": xh,
            "wq": wq, "wk": wk, "wv": wv,
            "bq": bqh, "bk": bkh, "bvb": bvb,
        })
    return in_maps


def run(inputs, trace=False, tmpdir=None):
    nc = build()
    in_maps = make_in_maps(**inputs)
    kw = {}
    if tmpdir is not None:
        import os
        os.makedirs(tmpdir, exist_ok=True)
        kw["tmpdir"] = tmpdir
    res = run_bass_kernel_spmd(
        nc, in_maps, core_ids=list(range(N_CORES)), trace=trace, **kw,
    )
    out = np.empty((B, N, D_OUT), dtype=np.float32)
    for c in range(N_CORES):
        b, qh = divmod(c, 2)
        out[b, qh * NQ:(qh + 1) * NQ, :] = res.results[c]["out"]
    return out, res.exec_time_ns


def kernel(**inputs) -> np.ndarray:
    out, _ = run(inputs, trace=False)
    return out


# revision 11
# speedup vs baseline: 1.1531x; 1.1531x over previous
"""Trainium2 Bass kernel: batched self-attention layer (B=4, N=2048, D=512).

Reference (per batch b):
    xt = x[b].T                # x[b] is [D_IN, N] -> xt [N, D_IN]
    Q = xt @ Wq + bq;  K = xt @ Wk + bk;  V = xt @ Wv + bv
    S = Q @ K.T / sqrt(D)
    out[b] = softmax(S, -1) @ V

Sharding: 8 cores, core c -> batch b=c//2, query rows [qh*1024,(qh+1)*1024),
qh=c%2. Each core computes K/V for the full 2048 keys of its batch
(duplicated across the pair; no collectives).

Layout strategy (zero transposes anywhere):
  - x[b] arrives [D_IN, N]: natural lhsT/rhs layouts for everything.
  - Q^T [d, n]  = Wq.T @ x       (lhsT=Wq chunk, rhs=x chunk)
  - K^T [d, n]  = Wk.T @ x
  - V   [m, d]  = x.T @ Wv       (lhsT=x chunk,  rhs=Wv chunk)
  - S^T [m, q]  = K @ Q^T        (lhsT=K^T chunk, rhs=Q^T)
  - E = exp(S^T * scale) on ACT (scores are O(1); no max-subtraction needed)
  - out [q, d]  = E^T.T @ V      (lhsT=E chunk, rhs=V chunk), PSUM-accumulated
  - denom [q,1] = E^T.T @ ones   (same stationary operand, free-dim-1 matmul)
  - out = out * (1/denom) + bv   (bv commutes past normalization since the
    softmax weights sum to 1; bv broadcast tile built once via a K=1 matmul)
All matmul inputs are bitcast to float32r (full-rate PE at free dim >= 256).
"""

import math

import numpy as np

import concourse.bass as bass
import concourse.tile as tile
from concourse import bacc, mybir
from concourse.bass_utils import run_bass_kernel_spmd

P = 128
B, D_IN, N, D_OUT = 4, 512, 2048, 512
NQ = N // 2          # query rows per core
QS = 512             # query rows per attention slice
KC = D_IN // P       # contraction chunks for projections
DC = D_OUT // P      # output-feature chunks
MT = N // P          # key tiles
N_CORES = 8
F32 = mybir.dt.float32
FR = mybir.dt.float32r
BF = mybir.dt.bfloat16
SCALE = 1.0 / math.sqrt(float(D_OUT))

_CACHE = {}


def _emit(tc, nc, d):
    with (
        tc.tile_pool(name="pers", bufs=1) as pers,
        tc.tile_pool(name="work", bufs=3) as work,
        tc.tile_pool(name="ps_proj", bufs=2, space="PSUM") as ps_proj,
        tc.tile_pool(name="ps_sc", bufs=2, space="PSUM") as ps_sc,
        tc.tile_pool(name="ps_out", bufs=4, space="PSUM") as ps_out,
    ):
        # ---- persistent loads ----
        # x arrives host-permuted: this core's query half occupies columns
        # [0, NQ) (so Q-projection slices x directly); key order is permuted,
        # which softmax/out sums are invariant to.
        # DMA order: wq -> x first halves -> wk -> x second halves -> wv, so
        # Q-projection matmuls start ~3us in and overlap the rest of the load.
        w_sb = {}

        def load_w(wn):
            for c in range(KC):
                t = pers.tile([P, D_OUT], BF, tag=f"{wn}{c}", name=f"{wn}{c}")
                nc.sync.dma_start(t, d[wn][:, c, :])
                w_sb[wn, c] = t

        load_w("wq")
        x_sb = []
        for c in range(KC):
            t = pers.tile([P, N], BF, tag=f"x{c}", name=f"x{c}")
            nc.sync.dma_start(t[:, 0:NQ], d["x"][:, c, 0:NQ])
            x_sb.append(t)
        bq_sb = pers.tile([P, KC], F32, tag="bq", name="bq_sb")
        nc.sync.dma_start(bq_sb, d["bq"][:, :])
        load_w("wk")
        for c in range(KC):
            nc.sync.dma_start(x_sb[c][:, NQ:N], d["x"][:, c, NQ:N])
        bk_sb = pers.tile([P, KC], F32, tag="bk", name="bk_sb")
        nc.sync.dma_start(bk_sb, d["bk"][:, :])
        load_w("wv")
        xq_sb = x_sb
        ones_m = pers.tile([P, 2], BF, tag="ones_m", name="ones_m")
        nc.vector.memset(ones_m, 1.0)
        zeros_8 = pers.tile([P, 8], BF, tag="zeros_8", name="zeros_8")
        nc.vector.memset(zeros_8, 0.0)
        bv_bc = pers.tile([P, D_OUT], F32, tag="bv_bc", name="bv_bc")
        nc.sync.dma_start(bv_bc, d["bvb"][:, :])

        # ---- projections ----
        qt_sb = [pers.tile([P, NQ], BF, tag=f"qt{i}", name=f"qt{i}") for i in range(DC)]
        for i in range(DC):
            for s in range(NQ // 512):
                ps = ps_proj.tile([P, 512], F32, tag="pp", name="ps_q")
                for c in range(KC):
                    nc.tensor.matmul(
                        ps,
                        lhsT=(w_sb["wq", c][:, i * P:(i + 1) * P]),
                        rhs=(xq_sb[c][:, s * 512:(s + 1) * 512]),
                        start=(c == 0), stop=(c == KC - 1),
                    )
                nc.vector.tensor_scalar_add(
                    qt_sb[i][:, s * 512:(s + 1) * 512], ps, bq_sb[:, i:i + 1]
                )

        kt_sb = [pers.tile([P, N], BF, tag=f"kt{i}", name=f"kt{i}") for i in range(DC)]
        for i in range(DC):
            for s in range(N // 512):
                ps = ps_proj.tile([P, 512], F32, tag="pp", name="ps_k")
                for c in range(KC):
                    nc.tensor.matmul(
                        ps,
                        lhsT=(w_sb["wk", c][:, i * P:(i + 1) * P]),
                        rhs=(x_sb[c][:, s * 512:(s + 1) * 512]),
                        start=(c == 0), stop=(c == KC - 1),
                    )
                nc.vector.tensor_scalar_add(
                    kt_sb[i][:, s * 512:(s + 1) * 512], ps, bk_sb[:, i:i + 1]
                )

        v_sb = [pers.tile([P, D_OUT], BF, tag=f"v{m}", name=f"v{m}") for m in range(MT)]
        for m in range(MT):
            ps = ps_proj.tile([P, D_OUT], F32, tag="pp", name="ps_v")
            for c in range(KC):
                nc.tensor.matmul(
                    ps,
                    lhsT=(x_sb[c][:, m * P:(m + 1) * P]),
                    rhs=(w_sb["wv", c]),
                    start=(c == 0), stop=(c == KC - 1),
                )
            nc.vector.tensor_copy(v_sb[m], ps)

        # ---- attention ----
        for s in range(NQ // QS):
            outp = [ps_out.tile([P, D_OUT], F32, tag="po", name=f"ps_o{q}") for q in range(QS // P)]
            dnp = ps_proj.tile([P, 2 * (QS // P)], F32, tag="pp", name="ps_dn")
            # one start=True matmul zeroes the whole denominator bank; the
            # per-q groups below accumulate with start=False (a start=True per
            # group would mark the full 2KB zero-region pending-zero and wipe
            # its bank-mates)
            nc.tensor.matmul(
                dnp, lhsT=(kt_sb[0][:, 0:P]), rhs=(zeros_8),
                start=True, stop=False, skip_group_check=True,
            )
            for m in range(MT):
                sps = ps_sc.tile([P, QS], F32, tag="ps", name="ps_s")
                for i in range(DC):
                    nc.tensor.matmul(
                        sps,
                        lhsT=(kt_sb[i][:, m * P:(m + 1) * P]),
                        rhs=(qt_sb[i][:, s * QS:(s + 1) * QS]),
                        start=(i == 0), stop=(i == DC - 1),
                    )
                e_sb = work.tile([P, QS], BF, tag="e", name="e_sb")
                nc.scalar.activation(
                    e_sb, sps, mybir.ActivationFunctionType.Exp, scale=SCALE
                )
                for q in range(QS // P):
                    nc.tensor.matmul(
                        outp[q],
                        lhsT=(e_sb[:, q * P:(q + 1) * P]),
                        rhs=(v_sb[m]),
                        start=(m == 0), stop=(m == MT - 1),
                    )
                    nc.tensor.matmul(
                        dnp[:, 2 * q:2 * q + 2],
                        lhsT=(e_sb[:, q * P:(q + 1) * P]),
                        rhs=(ones_m),
                        start=False, stop=(m == MT - 1),
                        skip_group_check=True,
                    )
            for q in range(QS // P):
                rc = work.tile([P, 1], F32, tag="rc", name="rc")
                nc.vector.reciprocal(rc, dnp[:, 2 * q:2 * q + 1])
                o_sb = work.tile([P, D_OUT], F32, tag="o", name="o_sb")
                nc.vector.scalar_tensor_tensor(
                    o_sb, outp[q], rc, bv_bc,
                    op0=mybir.AluOpType.mult, op1=mybir.AluOpType.add,
                )
                q0 = s * QS + q * P
                nc.sync.dma_start(d["out"][q0:q0 + P, :], o_sb)


def build():
    if "nc" in _CACHE:
        return _CACHE["nc"]
    nc = bacc.Bacc("TRN2", target_bir_lowering=False, debug=False,
                   num_devices=N_CORES)
    d = {
        "x": nc.declare_dram_parameter("x", [P, KC, N], BF, isOutput=False),
        "wq": nc.declare_dram_parameter("wq", [P, KC, D_OUT], BF, isOutput=False),
        "wk": nc.declare_dram_parameter("wk", [P, KC, D_OUT], BF, isOutput=False),
        "wv": nc.declare_dram_parameter("wv", [P, KC, D_OUT], BF, isOutput=False),
        "bq": nc.declare_dram_parameter("bq", [P, KC], F32, isOutput=False),
        "bk": nc.declare_dram_parameter("bk", [P, KC], F32, isOutput=False),
        "bvb": nc.declare_dram_parameter("bvb", [P, D_OUT], F32, isOutput=False),
        "out": nc.declare_dram_parameter("out", [NQ, D_OUT], F32, isOutput=True),
    }
    with tile.TileContext(nc) as tc:
        _emit(tc, nc, d)
    nc.compile()
    _CACHE["nc"] = nc
    return nc


def _f32(a):
    return np.ascontiguousarray(np.asarray(a), dtype=np.float32)


def _bf16(a):
    import ml_dtypes
    return np.ascontiguousarray(np.asarray(a, dtype=np.float32).astype(ml_dtypes.bfloat16))


def make_in_maps(x, Wq, bq, Wk, bk, Wv, bv):
    x = _f32(x)
    # [in, out] weights -> [128, 4, 512]: w[p, c, o] = W[c*128+p, o]
    wq = _bf16(_f32(Wq).reshape(KC, P, D_OUT).transpose(1, 0, 2))
    wk = _bf16(_f32(Wk).reshape(KC, P, D_OUT).transpose(1, 0, 2))
    wv = _bf16(_f32(Wv).reshape(KC, P, D_OUT).transpose(1, 0, 2))
    bqh = _f32(bq).reshape(KC, P).T.copy()   # [128, 4]: b[c*128+p] at [p, c]
    bkh = _f32(bk).reshape(KC, P).T.copy()
    bvb = np.tile(_f32(bv).reshape(1, D_OUT), (P, 1)).copy()
    in_maps = []
    for c in range(N_CORES):
        b, qh = divmod(c, 2)
        xb = x[b]                                         # [512, 2048]
        if qh == 1:                                       # query half first
            xb = np.concatenate([xb[:, NQ:], xb[:, :NQ]], axis=1)
        xh = _bf16(xb.reshape(KC, P, N).transpose(1, 0, 2))
        in_maps.append({
            "x": xh,
            "wq": wq, "wk": wk, "wv": wv,
            "bq": bqh, "bk": bkh, "bvb": bvb,
        })
    return in_maps


def run(inputs, trace=False, tmpdir=None):
    nc = build()
    in_maps = make_in_maps(**inputs)
    kw = {}
    if tmpdir is not None:
        import os
        os.makedirs(tmpdir, exist_ok=True)
        kw["tmpdir"] = tmpdir
    from concourse.compiler_utils import temporarily_append_compiler_flags
    with temporarily_append_compiler_flags([
        "--internal-backend-options=--enable-neff-debug-info=true "
        "--dump-on-error --enable-ldw-opt=true "
        "--assign-static-dmas-to-sp=false",
    ]):
        res = run_bass_kernel_spmd(
            nc, in_maps, core_ids=list(range(N_CORES)), trace=trace, **kw,
        )
    out = np.empty((B, N, D_OUT), dtype=np.float32)
    for c in range(N_CORES):
        b, qh = divmod(c, 2)
        out[b, qh * NQ:(qh + 1) * NQ, :] = res.results[c]["out"]
    return out, res.exec_time_ns


def kernel(**inputs) -> np.ndarray:
    out, _ = run(inputs, trace=False)
    return out


# revision 12
# speedup vs baseline: 1.3659x; 1.1846x over previous
"""Trainium2 Bass kernel: batched self-attention layer (B=4, N=2048, D=512).

Reference (per batch b):
    xt = x[b].T                # x[b] is [D_IN, N] -> xt [N, D_IN]
    Q = xt @ Wq + bq;  K = xt @ Wk + bk;  V = xt @ Wv + bv
    S = Q @ K.T / sqrt(D)
    out[b] = softmax(S, -1) @ V

Sharding: 8 cores, core c -> batch b=c//2, query rows [qh*1024,(qh+1)*1024),
qh=c%2. Each core computes K/V for the full 2048 keys of its batch
(duplicated across the pair; no collectives).

Layout strategy (zero transposes anywhere):
  - x[b] arrives [D_IN, N]: natural lhsT/rhs layouts for everything.
  - Q^T [d, n]  = Wq.T @ x       (lhsT=Wq chunk, rhs=x chunk)
  - K^T [d, n]  = Wk.T @ x
  - V   [m, d]  = x.T @ Wv       (lhsT=x chunk,  rhs=Wv chunk)
  - S^T [m, q]  = K @ Q^T        (lhsT=K^T chunk, rhs=Q^T)
  - E = exp(S^T * scale) on ACT (scores are O(1); no max-subtraction needed)
  - out [q, d]  = E^T.T @ V      (lhsT=E chunk, rhs=V chunk), PSUM-accumulated
  - denom [q,1] = E^T.T @ ones   (same stationary operand, free-dim-1 matmul)
  - out = out * (1/denom) + bv   (bv commutes past normalization since the
    softmax weights sum to 1; bv broadcast tile built once via a K=1 matmul)
All matmul inputs are bitcast to float32r (full-rate PE at free dim >= 256).
"""

import math

import numpy as np

import concourse.bass as bass
import concourse.tile as tile
from concourse import bacc, mybir
from concourse.bass_utils import run_bass_kernel_spmd

P = 128
B, D_IN, N, D_OUT = 4, 512, 2048, 512
NQ = N // 2          # query rows per core
QS = 512             # query rows per attention slice
KC = D_IN // P       # contraction chunks for projections
DC = D_OUT // P      # output-feature chunks
MT = N // P          # key tiles
N_CORES = 8
F32 = mybir.dt.float32
FR = mybir.dt.float32r
BF = mybir.dt.bfloat16
SCALE = 1.0 / math.sqrt(float(D_OUT))

_CACHE = {}


def _emit(tc, nc, d):
    with (
        tc.tile_pool(name="pers", bufs=1) as pers,
        tc.tile_pool(name="work", bufs=3) as work,
        tc.tile_pool(name="ps_proj", bufs=2, space="PSUM") as ps_proj,
        tc.tile_pool(name="ps_sc", bufs=2, space="PSUM") as ps_sc,
        tc.tile_pool(name="ps_out", bufs=4, space="PSUM") as ps_out,
    ):
        # ---- persistent loads ----
        # x arrives host-permuted: this core's query half occupies columns
        # [0, NQ) (so Q-projection slices x directly); key order is permuted,
        # which softmax/out sums are invariant to.
        # DMA order: wq -> x first halves -> wk -> x second halves -> wv, so
        # Q-projection matmuls start ~3us in and overlap the rest of the load.
        w_sb = {}

        def load_w(wn):
            for c in range(KC):
                t = pers.tile([P, D_OUT], BF, tag=f"{wn}{c}", name=f"{wn}{c}")
                nc.sync.dma_start(t, d[wn][:, c, :])
                w_sb[wn, c] = t

        load_w("wq")
        x_sb = []
        for c in range(KC):
            t = pers.tile([P, N], BF, tag=f"x{c}", name=f"x{c}")
            nc.sync.dma_start(t[:, 0:NQ], d["x"][:, c, 0:NQ])
            x_sb.append(t)
        bq_sb = pers.tile([P, KC], F32, tag="bq", name="bq_sb")
        nc.sync.dma_start(bq_sb, d["bq"][:, :])
        bk_sb = pers.tile([P, KC], F32, tag="bk", name="bk_sb")
        nc.sync.dma_start(bk_sb, d["bk"][:, :])
        load_w("wk")
        load_w("wv")
        for c in range(KC):
            nc.sync.dma_start(x_sb[c][:, NQ:N], d["x"][:, c, NQ:N])
        xq_sb = x_sb
        ones_m = pers.tile([P, 2], BF, tag="ones_m", name="ones_m")
        nc.vector.memset(ones_m, 1.0)
        zeros_8 = pers.tile([P, 8], BF, tag="zeros_8", name="zeros_8")
        nc.vector.memset(zeros_8, 0.0)
        bv_bc = pers.tile([P, D_OUT], F32, tag="bv_bc", name="bv_bc")
        nc.sync.dma_start(bv_bc, d["bvb"][:, :])

        # ---- projections ----
        qt_sb = [pers.tile([P, NQ], BF, tag=f"qt{i}", name=f"qt{i}") for i in range(DC)]
        for i in range(DC):
            for s in range(NQ // 512):
                ps = ps_proj.tile([P, 512], F32, tag="pp", name="ps_q")
                for c in range(KC):
                    nc.tensor.matmul(
                        ps,
                        lhsT=(w_sb["wq", c][:, i * P:(i + 1) * P]),
                        rhs=(xq_sb[c][:, s * 512:(s + 1) * 512]),
                        start=(c == 0), stop=(c == KC - 1),
                    )
                nc.vector.tensor_scalar_add(
                    qt_sb[i][:, s * 512:(s + 1) * 512], ps, bq_sb[:, i:i + 1]
                )

        kt_sb = [pers.tile([P, N], BF, tag=f"kt{i}", name=f"kt{i}") for i in range(DC)]
        v_sb = [pers.tile([P, D_OUT], BF, tag=f"v{m}", name=f"v{m}") for m in range(MT)]

        def k_proj(i, s):
            ps = ps_proj.tile([P, 512], F32, tag="pp", name="ps_k")
            for c in range(KC):
                nc.tensor.matmul(
                    ps,
                    lhsT=(w_sb["wk", c][:, i * P:(i + 1) * P]),
                    rhs=(x_sb[c][:, s * 512:(s + 1) * 512]),
                    start=(c == 0), stop=(c == KC - 1),
                )
            nc.vector.tensor_scalar_add(
                kt_sb[i][:, s * 512:(s + 1) * 512], ps, bk_sb[:, i:i + 1]
            )

        def v_proj(m):
            ps = ps_proj.tile([P, D_OUT], F32, tag="pp", name="ps_v")
            for c in range(KC):
                nc.tensor.matmul(
                    ps,
                    lhsT=(x_sb[c][:, m * P:(m + 1) * P]),
                    rhs=(w_sb["wv", c]),
                    start=(c == 0), stop=(c == KC - 1),
                )
            nc.vector.tensor_copy(v_sb[m], ps)

        # first-half K/V only touch x columns [0, NQ) (already resident);
        # second-half runs after the xh2 DMAs land
        for i in range(DC):
            for s in range(2):
                k_proj(i, s)
        for m in range(MT // 2):
            v_proj(m)
        for i in range(DC):
            for s in range(2, 4):
                k_proj(i, s)
        for m in range(MT // 2, MT):
            v_proj(m)

        # ---- attention ----
        for s in range(NQ // QS):
            outp = [ps_out.tile([P, D_OUT], F32, tag="po", name=f"ps_o{q}") for q in range(QS // P)]
            dnp = ps_proj.tile([P, 2 * (QS // P)], F32, tag="pp", name="ps_dn")
            # one start=True matmul zeroes the whole denominator bank; the
            # per-q groups below accumulate with start=False (a start=True per
            # group would mark the full 2KB zero-region pending-zero and wipe
            # its bank-mates)
            nc.tensor.matmul(
                dnp, lhsT=(kt_sb[0][:, 0:P]), rhs=(zeros_8),
                start=True, stop=False, skip_group_check=True,
            )
            for m in range(MT):
                sps = ps_sc.tile([P, QS], F32, tag="ps", name="ps_s")
                for i in range(DC):
                    nc.tensor.matmul(
                        sps,
                        lhsT=(kt_sb[i][:, m * P:(m + 1) * P]),
                        rhs=(qt_sb[i][:, s * QS:(s + 1) * QS]),
                        start=(i == 0), stop=(i == DC - 1),
                    )
                e_sb = work.tile([P, QS], BF, tag="e", name="e_sb")
                nc.scalar.activation(
                    e_sb, sps, mybir.ActivationFunctionType.Exp, scale=SCALE
                )
                for q in range(QS // P):
                    nc.tensor.matmul(
                        outp[q],
                        lhsT=(e_sb[:, q * P:(q + 1) * P]),
                        rhs=(v_sb[m]),
                        start=(m == 0), stop=(m == MT - 1),
                    )
                    nc.tensor.matmul(
                        dnp[:, 2 * q:2 * q + 2],
                        lhsT=(e_sb[:, q * P:(q + 1) * P]),
                        rhs=(ones_m),
                        start=False, stop=(m == MT - 1),
                        skip_group_check=True,
                    )
            rc = work.tile([P, 2 * (QS // P)], F32, tag="rc", name="rc")
            nc.vector.reciprocal(rc, dnp)
            for q in range(QS // P):
                o_sb = work.tile([P, D_OUT], F32, tag="o", name="o_sb")
                nc.vector.scalar_tensor_tensor(
                    o_sb, outp[q], rc[:, 2 * q:2 * q + 1], bv_bc,
                    op0=mybir.AluOpType.mult, op1=mybir.AluOpType.add,
                )
                q0 = s * QS + q * P
                nc.sync.dma_start(d["out"][q0:q0 + P, :], o_sb)


def build():
    if "nc" in _CACHE:
        return _CACHE["nc"]
    nc = bacc.Bacc("TRN2", target_bir_lowering=False, debug=False,
                   num_devices=N_CORES)
    d = {
        "x": nc.declare_dram_parameter("x", [P, KC, N], BF, isOutput=False),
        "wq": nc.declare_dram_parameter("wq", [P, KC, D_OUT], BF, isOutput=False),
        "wk": nc.declare_dram_parameter("wk", [P, KC, D_OUT], BF, isOutput=False),
        "wv": nc.declare_dram_parameter("wv", [P, KC, D_OUT], BF, isOutput=False),
        "bq": nc.declare_dram_parameter("bq", [P, KC], F32, isOutput=False),
        "bk": nc.declare_dram_parameter("bk", [P, KC], F32, isOutput=False),
        "bvb": nc.declare_dram_parameter("bvb", [P, D_OUT], F32, isOutput=False),
        "out": nc.declare_dram_parameter("out", [NQ, D_OUT], F32, isOutput=True),
    }
    with tile.TileContext(nc) as tc:
        _emit(tc, nc, d)
    nc.compile()
    _CACHE["nc"] = nc
    return nc


def _f32(a):
    return np.ascontiguousarray(np.asarray(a), dtype=np.float32)


def _bf16(a):
    import ml_dtypes
    return np.ascontiguousarray(np.asarray(a, dtype=np.float32).astype(ml_dtypes.bfloat16))


def make_in_maps(x, Wq, bq, Wk, bk, Wv, bv):
    x = _f32(x)
    # [in, out] weights -> [128, 4, 512]: w[p, c, o] = W[c*128+p, o]
    wq = _bf16(_f32(Wq).reshape(KC, P, D_OUT).transpose(1, 0, 2))
    wk = _bf16(_f32(Wk).reshape(KC, P, D_OUT).transpose(1, 0, 2))
    wv = _bf16(_f32(Wv).reshape(KC, P, D_OUT).transpose(1, 0, 2))
    bqh = _f32(bq).reshape(KC, P).T.copy()   # [128, 4]: b[c*128+p] at [p, c]
    bkh = _f32(bk).reshape(KC, P).T.copy()
    bvb = np.tile(_f32(bv).reshape(1, D_OUT), (P, 1)).copy()
    in_maps = []
    for c in range(N_CORES):
        b, qh = divmod(c, 2)
        xb = x[b]                                         # [512, 2048]
        if qh == 1:                                       # query half first
            xb = np.concatenate([xb[:, NQ:], xb[:, :NQ]], axis=1)
        xh = _bf16(xb.reshape(KC, P, N).transpose(1, 0, 2))
        in_maps.append({
            "x": xh,
            "wq": wq, "wk": wk, "wv": wv,
            "bq": bqh, "bk": bkh, "bvb": bvb,
        })
    return in_maps


def run(inputs, trace=False, tmpdir=None):
    nc = build()
    in_maps = make_in_maps(**inputs)
    kw = {}
    if tmpdir is not None:
        import os
        os.makedirs(tmpdir, exist_ok=True)
        kw["tmpdir"] = tmpdir
    from concourse.compiler_utils import temporarily_append_compiler_flags
    with temporarily_append_compiler_flags([
        "--internal-backend-options=--enable-neff-debug-info=true "
        "--dump-on-error --enable-ldw-opt=true "
        "--assign-static-dmas-to-sp=false",
    ]):
        res = run_bass_kernel_spmd(
            nc, in_maps, core_ids=list(range(N_CORES)), trace=trace, **kw,
        )
    out = np.empty((B, N, D_OUT), dtype=np.float32)
    for c in range(N_CORES):
        b, qh = divmod(c, 2)
        out[b, qh * NQ:(qh + 1) * NQ, :] = res.results[c]["out"]
    return out, res.exec_time_ns


def kernel(**inputs) -> np.ndarray:
    out, _ = run(inputs, trace=False)
    return out
